# revision 21
# baseline (speedup 1.0000x reference)
"""MLA prefill attention (DeepSeek-style), tensor-parallel over heads on 8 TRN2 NeuronCores.

Reference computation (per head h, per batch b of 4 x 1024 tokens):
  kv_c   = k[:, 0, :512]                  # [N, 512] compressed latent (shared)
  k_nope = kv_c @ w_key[h].T              # [N, 128]
  k_full = concat(k_nope, k_rope)         # [N, 192]
  v_raw  = kv_c @ w_vo[h].T               # [N, 128]
  o      = softmax(causal(q_h @ k_full.T * SCALE)) @ v_raw

Sharding: 16 heads / 8 cores = 2 heads per core; kv_c replicated. No collectives.

Device kernel (per core, all matmuls bf16):
  Phase 1: k_nopeT [128d, N] per head; v_raw for BOTH heads per 128-token
    chunk in one 256-wide matmul (hides weight loads), stored interleaved as
    [v_h0 | 1 | v_h1 | 1] per chunk so each head's PV slice is contiguous
    [128,129] with a ones column.
  Phase 2: transposed-score flash attention: scoresT [k, q] = k_fullT.T @ qT,
    exp on ScalarE (softmax scale folded in; no max pass needed, scores are
    O(5) bounded), causal triangle masked multiplicatively on the first 128
    cols of diagonal chunks only (columns beyond the diagonal block are fully
    valid; trapezoid tiling skips the fully-masked region). PV uses probsT
    blocks as the STATIONARY operand and v_aug as moving, accumulating
    non-transposed o[q, dv] in PSUM with the softmax denominator in column
    128. Accumulators are bank-packed in pairs so two q-blocks pipeline.
    Epilogue: reciprocal + per-partition scaled copy on DVE, then one DMA
    per q-block on the second HWDGE ring.

  All large DMAs are single contiguous >=1MB transfers (host pre-packs the
  layouts); q loads are emitted after phase 1 so kv wins the DMA queue.
"""

import os
import sys

sys.path.insert(0, "/opt/trn_rl_repo")

from contextlib import ExitStack

import numpy as np
import ml_dtypes

import concourse.bass as bass
import concourse.mybir as mybir
from concourse import bacc, tile
from concourse.bass_utils import run_bass_kernel_spmd

B, S, H, N = 4, 1024, 16, 4096
DN, DR, DV, R = 128, 64, 128, 512
SCALE = 0.07216878364870323
NCORES = 8
HPC = H // NCORES  # heads per core
P = 128
QBLK = 512
NRC = R // P  # 4 r-chunks
NBLK = 8      # kv column blocks (DMA pipelining granularity)
BCOLS = N // NBLK
DVA = DV + 1   # v | ones  -> rowsums fall out of PV
VCH = 2 * DVA  # combined both-heads v chunk stride [v0 | 1 | v1 | 1]
BF16 = mybir.dt.bfloat16
F8 = mybir.dt.float8e4
F32 = mybir.dt.float32
Exp = mybir.ActivationFunctionType.Exp
EXP_BIAS = -2.5  # shift-invariant softmax bias keeps exp outputs << fp8e4 max (240)

_CACHE: dict = {}


def _build():
    nc = bacc.Bacc("TRN2", target_bir_lowering=False, debug=False, num_devices=NCORES)

    qtn = nc.dram_tensor("qtn", [HPC, DN, N], BF16, kind="ExternalInput").ap()
    qtr = nc.dram_tensor("qtr", [HPC, DR, N], BF16, kind="ExternalInput").ap()
    # per column-block, r-chunks side by side: [blk][128r, c*BCOLS + n]
    kvt = nc.dram_tensor("kvt", [NBLK, P, NRC * BCOLS], BF16,
                         kind="ExternalInput").ap()
    krt = nc.dram_tensor("krt", [DR, N], BF16, kind="ExternalInput").ap()
    # w_key per head, r-chunks side by side: [h][128r, c*DN + d]
    wkt = nc.dram_tensor("wkt", [HPC, P, NRC * DN], BF16, kind="ExternalInput").ap()
    # w_vo both heads per r-chunk: [c][128r, h*DV + d]
    wvt = nc.dram_tensor("wvt", [NRC, P, HPC * DV], BF16, kind="ExternalInput").ap()
    mskd = nc.dram_tensor("mskd", [P, P], F32, kind="ExternalInput").ap()
    out = nc.dram_tensor("out", [HPC, N, DV], F32, kind="ExternalOutput").ap()

    with tile.TileContext(nc) as tc, ExitStack() as ctx:
        const = ctx.enter_context(tc.tile_pool(name="const", bufs=1))
        res = ctx.enter_context(tc.tile_pool(name="res", bufs=1))
        prs = ctx.enter_context(tc.tile_pool(name="prs", bufs=8))
        osb = ctx.enter_context(tc.tile_pool(name="osb", bufs=6))
        psA = ctx.enter_context(tc.tile_pool(name="psA", bufs=4, space="PSUM"))
        psO = ctx.enter_context(tc.tile_pool(name="psO", bufs=4, space="PSUM"))

        msk = const.tile([P, P], F32)
        nc.sync.dma_start(msk[:], mskd[:])
        ebias = const.tile([P, 1], F32)
        nc.gpsimd.memset(ebias[:], EXP_BIAS)

        # DMA order minimizes time-to-first-matmul: block 0's kv and w_vo
        # are split per r-chunk and interleaved so the first v-build
        # accumulation starts after ~192KB instead of ~1MB; later blocks
        # stay single 1MB contiguous transfers. FIFO ring per engine.
        wv_sb, kv0_sb = [], []
        for c in range(NRC):
            t = res.tile([P, HPC * DV], BF16, tag=f"wv{c}", name=f"wv{c}")
            nc.sync.dma_start(t[:], wvt[c])
            wv_sb.append(t)
            t = res.tile([P, BCOLS], BF16, tag=f"kv0_{c}", name=f"kv0_{c}")
            nc.sync.dma_start(t[:], kvt[0, :, c * BCOLS:(c + 1) * BCOLS])
            kv0_sb.append(t)
        wk_sb = []
        for h in range(HPC):
            t = res.tile([P, NRC * DN], BF16, tag=f"wk{h}", name=f"wk{h}")
            nc.sync.dma_start(t[:], wkt[h])
            wk_sb.append(t)

        kv_sb = [None]
        for blk in range(1, NBLK):
            t = res.tile([P, NRC * BCOLS], BF16, tag=f"kv{blk}", name=f"kv{blk}")
            nc.sync.dma_start(t[:], kvt[blk])
            kv_sb.append(t)

        def kv(c, blk, lo, hi):  # cols [lo,hi) of r-chunk c within block blk
            if blk == 0:
                return kv0_sb[c][:, lo:hi]
            return kv_sb[blk][:, c * BCOLS + lo:c * BCOLS + hi]

        kr_sb = res.tile([DR, N], BF16)
        nc.sync.dma_start(kr_sb[:], krt[:])

        kn_sb = [
            res.tile([P, N], BF16, tag=f"kn{h}", name=f"kn{h}") for h in range(HPC)
        ]
        # combined v_aug for both heads; chunk ki at [:, ki*VCH : (ki+1)*VCH]
        # = [v_h0(128) | 1 | v_h1(128) | 1]; memset pre-fills the ones cols.
        vcomb = res.tile([P, (N // P) * VCH], BF16)
        nc.gpsimd.memset(vcomb[:], 1.0)

        # ---- Phase 1 ----
        for blk in range(NBLK):
            # v for both heads: out [128n, 256] = kv_chunk.T @ [w_vo0|w_vo1]
            for ki in range(BCOLS // P):
                psv = psO.tile([P, HPC * DV], F32, tag="psO", name="vb")
                kg = blk * (BCOLS // P) + ki
                for c in range(NRC):
                    nc.tensor.matmul(
                        psv[:], lhsT=kv(c, blk, ki * P, (ki + 1) * P),
                        rhs=wv_sb[c][:],
                        start=(c == 0), stop=(c == NRC - 1),
                    )
                # [128, 2, 128] -> strided dest blocks at 0 and DVA
                dst = vcomb[:, kg * VCH:(kg + 1) * VCH]
                nc.vector.tensor_copy(
                    dst.rearrange("p (h d) -> p h d", h=HPC)[:, :, 0:DV],
                    psv[:].rearrange("p (h d) -> p h d", h=HPC),
                )
            for h in range(HPC):
                for j in range(BCOLS // QBLK):
                    ps = psA.tile([P, QBLK], F32, tag="psA", name="knb")
                    js = slice(blk * BCOLS + j * QBLK, blk * BCOLS + (j + 1) * QBLK)
                    for c in range(NRC):
                        nc.tensor.matmul(
                            ps[:], lhsT=wk_sb[h][:, c * DN:(c + 1) * DN],
                            rhs=kv(c, blk, j * QBLK, (j + 1) * QBLK),
                            start=(c == 0), stop=(c == NRC - 1),
                        )
                    nc.vector.tensor_copy(kn_sb[h][:, js], ps[:])

        # q DMAs emitted after phase 1 so the kv blocks phase 1 depends on
        # win the DMA queues; q is only needed once phase 2 starts.
        qn_sb, qr_sb = [], []
        for h in range(HPC):
            t = res.tile([DN, N], BF16, tag=f"qn{h}", name=f"qn{h}")
            nc.sync.dma_start(t[:], qtn[h])
            qn_sb.append(t)
            t = res.tile([DR, N], BF16, tag=f"qr{h}", name=f"qr{h}")
            nc.sync.dma_start(t[:], qtr[h])
            qr_sb.append(t)

        # ---- Phase 2: attention ----
        for h in range(HPC):
            for b in range(B):
                q0 = b * S
                for qb in range(S // QBLK):
                    qs = qb * QBLK
                    nfull = qs // P
                    kis = nfull + QBLK // P
                    # two bank-packed accumulator pairs:
                    # [o_j4(128) | rs | o_j4+1(128) | rs]
                    ops = [
                        psO.tile([P, 2 * DVA], F32, tag="psO", name=f"opair{p_}")
                        for p_ in range(2)
                    ]
                    def emit_pv(ki, j, qoff, pr):
                        kidx = (q0 + ki * P) // P
                        va = vcomb[:, kidx * VCH + h * DVA:kidx * VCH + h * DVA + DVA]
                        for j4 in range(max(0, j), QBLK // P):
                            # start=True clears has_written for the WHOLE
                            # bank, so only the first write of each bank-packed
                            # pair may use it; the partner's first matmul
                            # overwrites via the already-cleared bits.
                            nc.tensor.matmul(
                                ops[j4 // 2][:, (j4 % 2) * DVA:(j4 % 2 + 1) * DVA],
                                lhsT=pr[:, j4 * P - qoff:(j4 + 1) * P - qoff],
                                rhs=va,
                                start=(ki == 0 and j4 % 2 == 0),
                                stop=(ki == nfull + j4),
                                skip_group_check=True,
                            )

                    pending = None
                    for ki in range(kis):
                        if ki < nfull:
                            j, qoff, w = -1, 0, QBLK
                        else:
                            j = ki - nfull
                            qoff = j * P
                            w = QBLK - qoff
                        qg = q0 + qs + qoff
                        kg = q0 + ki * P
                        ks = slice(kg, kg + P)
                        sc = psA.tile([P, QBLK], F32, tag="psA", name="sc")
                        nc.tensor.matmul(
                            sc[:, :w], lhsT=kn_sb[h][:, ks],
                            rhs=qn_sb[h][:, qg:qg + w], start=True, stop=False,
                        )
                        nc.tensor.matmul(
                            sc[:, :w], lhsT=kr_sb[:, ks],
                            rhs=qr_sb[h][:, qg:qg + w], start=False, stop=True,
                        )
                        if ki >= nfull:
                            # additive causal mask on raw scores in PSUM; only
                            # the leading 128 cols contain the triangle, all
                            # columns beyond the diagonal block are valid
                            nc.vector.tensor_add(sc[:, 0:P], sc[:, 0:P], msk[:])
                        pr = prs.tile([P, QBLK], BF16, tag="probs", name="pr")
                        nc.scalar.activation(pr[:, :w], sc[:, :w], Exp,
                                             scale=SCALE, bias=ebias[:])
                        # software pipeline: emit PV for the PREVIOUS chunk so
                        # PE streams scores(ki) while ACT computes exp(ki-1)
                        if pending is not None:
                            emit_pv(*pending)
                        pending = (ki, j, qoff, pr)
                    emit_pv(*pending)
                    oq = osb.tile([P, QBLK], F32, tag="oq", name="oq")
                    for j4 in range(QBLK // P):
                        op = ops[j4 // 2]
                        off = (j4 % 2) * DVA
                        rec = osb.tile([P, 1], F32, tag="rec", name="rec")
                        nc.vector.reciprocal(rec[:], op[:, off + DV:off + DVA])
                        nc.vector.tensor_scalar_mul(
                            oq[:, j4 * P:(j4 + 1) * P], op[:, off:off + DV], rec[:]
                        )
                    # one 256KB DMA per q-block on the ACT HWDGE ring;
                    # oq[p, j4*128+d] <-> out[h, q0+qs+j4*128+p, d]
                    nc.scalar.dma_start(
                        out[h, q0 + qs:q0 + qs + QBLK, :]
                        .rearrange("(j p) d -> p j d", p=P),
                        oq[:].rearrange("p (j d) -> p j d", d=DV),
                    )

    nc.compile()
    return nc


def _prep_inputs(q, k, w_key, w_vo):
    bf = ml_dtypes.bfloat16
    kv_c = np.ascontiguousarray(k[:, 0, :R])          # [N, 512]
    k_rope = np.ascontiguousarray(k[:, 0, R:])        # [N, 64]
    # kvt[blk][rl, c*BCOLS+nl] = kv_c[blk*BCOLS+nl, c*128+rl]
    kvt = np.ascontiguousarray(
        kv_c.T.reshape(NRC, P, NBLK, BCOLS).transpose(2, 1, 0, 3)
        .reshape(NBLK, P, NRC * BCOLS).astype(bf))
    krt = np.ascontiguousarray(k_rope.T.astype(bf))   # [64, N]
    msk = np.where(np.triu(np.ones((P, P), bool)), 0.0, -1e5).astype(np.float32)

    in_maps = []
    for core in range(NCORES):
        hs = slice(core * HPC, (core + 1) * HPC)
        qh = q[:, hs, :]                              # [N, HPC, 192]
        qtn = np.ascontiguousarray(
            qh[:, :, :DN].transpose(1, 2, 0).astype(bf))   # [HPC, 128, N]
        qtr = np.ascontiguousarray(
            qh[:, :, DN:].transpose(1, 2, 0).astype(bf))   # [HPC, 64, N]
        # wkt[h][rl, c*DN+d] = w_key[hs][h, d, c*128+rl]
        wkt = np.ascontiguousarray(
            w_key[hs].transpose(0, 2, 1).reshape(HPC, NRC, P, DN)
            .transpose(0, 2, 1, 3).reshape(HPC, P, NRC * DN).astype(bf))
        # wvt[c][rl, h*DV + d] = w_vo[hs][h, d, c*128+rl]
        wvt = np.ascontiguousarray(
            w_vo[hs].transpose(2, 0, 1)               # [512r, HPC, 128d]
            .reshape(NRC, P, HPC * DV).astype(bf))
        in_maps.append({
            "qtn": qtn, "qtr": qtr, "kvt": kvt, "krt": krt,
            "wkt": wkt, "wvt": wvt, "mskd": msk,
        })
    return in_maps


def run(q, k, v, w_key, w_vo, trace=False, tmpdir=None):
    """Returns (output [N, H, 128] f32, BassKernelResults)."""
    if "nc" not in _CACHE:
        _CACHE["nc"] = _build()
    nc = _CACHE["nc"]
    in_maps = _prep_inputs(np.asarray(q), np.asarray(k),
                           np.asarray(w_key), np.asarray(w_vo))
    res = run_bass_kernel_spmd(
        nc, in_maps, core_ids=list(range(NCORES)), trace=trace, tmpdir=tmpdir
    )
    outs = [np.asarray(res.results[i]["out"], dtype=np.float32)
            for i in range(NCORES)]
    full = np.concatenate(outs, axis=0)               # [16, N, 128]
    return np.ascontiguousarray(full.transpose(1, 0, 2)), res


def kernel(q, k, v, w_key, w_vo):
    return run(q, k, v, w_key, w_vo)[0]


# revision 22
# speedup vs baseline: 1.3801x; 1.3801x over previous
"""MLA prefill attention (DeepSeek-style), tensor-parallel over heads on 8 TRN2 NeuronCores.

Reference computation (per head h, per batch b of 4 x 1024 tokens):
  kv_c   = k[:, 0, :512]                  # [N, 512] compressed latent (shared)
  k_nope = kv_c @ w_key[h].T              # [N, 128]
  k_full = concat(k_nope, k_rope)         # [N, 192]
  v_raw  = kv_c @ w_vo[h].T               # [N, 128]
  o      = softmax(causal(q_h @ k_full.T * SCALE)) @ v_raw

Sharding: 16 heads / 8 cores = 2 heads per core; kv_c replicated. No collectives.

Device kernel (per core, all matmuls bf16):
  Phase 1: k_nopeT [128d, N] per head; v_raw for BOTH heads per 128-token
    chunk in one 256-wide matmul (hides weight loads), stored interleaved as
    [v_h0 | 1 | v_h1 | 1] per chunk so each head's PV slice is contiguous
    [128,129] with a ones column.
  Phase 2: transposed-score flash attention: scoresT [k, q] = k_fullT.T @ qT,
    exp on ScalarE (softmax scale folded in; no max pass needed, scores are
    O(5) bounded), causal triangle masked multiplicatively on the first 128
    cols of diagonal chunks only (columns beyond the diagonal block are fully
    valid; trapezoid tiling skips the fully-masked region). PV uses probsT
    blocks as the STATIONARY operand and v_aug as moving, accumulating
    non-transposed o[q, dv] in PSUM with the softmax denominator in column
    128. Accumulators are bank-packed in pairs so two q-blocks pipeline.
    Epilogue: reciprocal + per-partition scaled copy on DVE, then one DMA
    per q-block on the second HWDGE ring.

  All large DMAs are single contiguous >=1MB transfers (host pre-packs the
  layouts); q loads are emitted after phase 1 so kv wins the DMA queue.
"""

import os
import sys

sys.path.insert(0, "/opt/trn_rl_repo")

from contextlib import ExitStack

import numpy as np
import ml_dtypes

import concourse.bass as bass
import concourse.mybir as mybir
from concourse import bacc, tile
from concourse.bass_utils import run_bass_kernel_spmd

B, S, H, N = 4, 1024, 16, 4096
DN, DR, DV, R = 128, 64, 128, 512
SCALE = 0.07216878364870323
NCORES = 8
HPC = H // NCORES  # heads per core
P = 128
QBLK = 512
NRC = R // P  # 4 r-chunks
NBLK = 8      # kv column blocks (DMA pipelining granularity)
BCOLS = N // NBLK
DVA = DV + 1   # v | ones  -> rowsums fall out of PV
VCH = 2 * DVA  # combined both-heads v chunk stride [v0 | 1 | v1 | 1]
BF16 = mybir.dt.bfloat16
F8 = mybir.dt.float8e4
F32 = mybir.dt.float32
Exp = mybir.ActivationFunctionType.Exp
EXP_BIAS = -2.5  # shift-invariant softmax bias keeps exp outputs << fp8e4 max (240)

_CACHE: dict = {}


def _build():
    nc = bacc.Bacc("TRN2", target_bir_lowering=False, debug=False, num_devices=NCORES)

    qtn = nc.dram_tensor("qtn", [HPC, DN, N], BF16, kind="ExternalInput").ap()
    qtr = nc.dram_tensor("qtr", [HPC, DR, N], BF16, kind="ExternalInput").ap()
    # per column-block, r-chunks side by side: [blk][128r, c*BCOLS + n]
    kvt = nc.dram_tensor("kvt", [NBLK, P, NRC * BCOLS], BF16,
                         kind="ExternalInput").ap()
    krt = nc.dram_tensor("krt", [DR, N], BF16, kind="ExternalInput").ap()
    # w_key per head, r-chunks side by side: [h][128r, c*DN + d]
    wkt = nc.dram_tensor("wkt", [HPC, P, NRC * DN], BF16, kind="ExternalInput").ap()
    # w_vo both heads per r-chunk: [c][128r, h*DV + d]
    wvt = nc.dram_tensor("wvt", [NRC, P, HPC * DV], BF16, kind="ExternalInput").ap()
    mskd = nc.dram_tensor("mskd", [P, P], F32, kind="ExternalInput").ap()
    out = nc.dram_tensor("out", [HPC, N, DV], F32, kind="ExternalOutput").ap()

    with tile.TileContext(nc) as tc, ExitStack() as ctx:
        const = ctx.enter_context(tc.tile_pool(name="const", bufs=1))
        res = ctx.enter_context(tc.tile_pool(name="res", bufs=1))
        prs = ctx.enter_context(tc.tile_pool(name="prs", bufs=8))
        osb = ctx.enter_context(tc.tile_pool(name="osb", bufs=6))
        psA = ctx.enter_context(tc.tile_pool(name="psA", bufs=4, space="PSUM"))
        psO = ctx.enter_context(tc.tile_pool(name="psO", bufs=4, space="PSUM"))

        msk = const.tile([P, P], F32)
        nc.sync.dma_start(msk[:], mskd[:])
        ebias = const.tile([P, 1], F32)
        nc.gpsimd.memset(ebias[:], EXP_BIAS)

        # DMA order minimizes time-to-first-matmul: block 0's kv and w_vo
        # are split per r-chunk and interleaved so the first v-build
        # accumulation starts after ~192KB instead of ~1MB; later blocks
        # stay single 1MB contiguous transfers. FIFO ring per engine.
        wv_sb, kv0_sb = [], []
        for c in range(NRC):
            t = res.tile([P, HPC * DV], BF16, tag=f"wv{c}", name=f"wv{c}")
            nc.sync.dma_start(t[:], wvt[c])
            wv_sb.append(t)
            t = res.tile([P, BCOLS], BF16, tag=f"kv0_{c}", name=f"kv0_{c}")
            nc.sync.dma_start(t[:], kvt[0, :, c * BCOLS:(c + 1) * BCOLS])
            kv0_sb.append(t)
        wk_sb = []
        for h in range(HPC):
            t = res.tile([P, NRC * DN], BF16, tag=f"wk{h}", name=f"wk{h}")
            nc.sync.dma_start(t[:], wkt[h])
            wk_sb.append(t)

        kv_sb = [None]
        for blk in range(1, NBLK):
            t = res.tile([P, NRC * BCOLS], BF16, tag=f"kv{blk}", name=f"kv{blk}")
            nc.sync.dma_start(t[:], kvt[blk])
            kv_sb.append(t)

        def kv(c, blk, lo, hi):  # cols [lo,hi) of r-chunk c within block blk
            if blk == 0:
                return kv0_sb[c][:, lo:hi]
            return kv_sb[blk][:, c * BCOLS + lo:c * BCOLS + hi]

        kr_sb = res.tile([DR, N], BF16)
        nc.sync.dma_start(kr_sb[:], krt[:])

        kn_sb = [
            res.tile([P, N], BF16, tag=f"kn{h}", name=f"kn{h}") for h in range(HPC)
        ]
        # combined v_aug for both heads; chunk ki at [:, ki*VCH : (ki+1)*VCH]
        # = [v_h0(128) | 1 | v_h1(128) | 1]; memset pre-fills the ones cols.
        vcomb = res.tile([P, (N // P) * VCH], BF16)
        nc.gpsimd.memset(vcomb[:], 1.0)

        # ---- Phase 1 ----
        for blk in range(NBLK):
            # v for both heads: out [128n, 256] = kv_chunk.T @ [w_vo0|w_vo1]
            for ki in range(BCOLS // P):
                psv = psO.tile([P, HPC * DV], F32, tag="psO", name="vb")
                kg = blk * (BCOLS // P) + ki
                for c in range(NRC):
                    nc.tensor.matmul(
                        psv[:], lhsT=kv(c, blk, ki * P, (ki + 1) * P),
                        rhs=wv_sb[c][:],
                        start=(c == 0), stop=(c == NRC - 1),
                    )
                # [128, 2, 128] -> strided dest blocks at 0 and DVA
                dst = vcomb[:, kg * VCH:(kg + 1) * VCH]
                nc.vector.tensor_copy(
                    dst.rearrange("p (h d) -> p h d", h=HPC)[:, :, 0:DV],
                    psv[:].rearrange("p (h d) -> p h d", h=HPC),
                )
            for h in range(HPC):
                for j in range(BCOLS // QBLK):
                    ps = psA.tile([P, QBLK], F32, tag="psA", name="knb")
                    js = slice(blk * BCOLS + j * QBLK, blk * BCOLS + (j + 1) * QBLK)
                    for c in range(NRC):
                        nc.tensor.matmul(
                            ps[:], lhsT=wk_sb[h][:, c * DN:(c + 1) * DN],
                            rhs=kv(c, blk, j * QBLK, (j + 1) * QBLK),
                            start=(c == 0), stop=(c == NRC - 1),
                        )
                    nc.scalar.copy(kn_sb[h][:, js], ps[:])

        # q DMAs emitted after phase 1 so the kv blocks phase 1 depends on
        # win the DMA queues; q is only needed once phase 2 starts.
        qn_sb, qr_sb = [], []
        for h in range(HPC):
            t = res.tile([DN, N], BF16, tag=f"qn{h}", name=f"qn{h}")
            nc.sync.dma_start(t[:], qtn[h])
            qn_sb.append(t)
            t = res.tile([DR, N], BF16, tag=f"qr{h}", name=f"qr{h}")
            nc.sync.dma_start(t[:], qtr[h])
            qr_sb.append(t)

        # ---- Phase 2: attention ----
        for h in range(HPC):
            for b in range(B):
                q0 = b * S
                for qb in range(S // QBLK):
                    qs = qb * QBLK
                    nfull = qs // P
                    kis = nfull + QBLK // P
                    # two bank-packed accumulator pairs:
                    # [o_j4(128) | rs | o_j4+1(128) | rs]
                    ops = [
                        psO.tile([P, 2 * DVA], F32, tag="psO", name=f"opair{p_}")
                        for p_ in range(2)
                    ]
                    def emit_pv(ki, j, qoff, pr):
                        kidx = (q0 + ki * P) // P
                        va = vcomb[:, kidx * VCH + h * DVA:kidx * VCH + h * DVA + DVA]
                        for j4 in range(max(0, j), QBLK // P):
                            # start=True clears has_written for the WHOLE
                            # bank, so only the first write of each bank-packed
                            # pair may use it; the partner's first matmul
                            # overwrites via the already-cleared bits.
                            nc.tensor.matmul(
                                ops[j4 // 2][:, (j4 % 2) * DVA:(j4 % 2 + 1) * DVA],
                                lhsT=pr[:, j4 * P - qoff:(j4 + 1) * P - qoff],
                                rhs=va,
                                start=(ki == 0 and j4 % 2 == 0),
                                stop=(ki == nfull + j4),
                                skip_group_check=True,
                            )

                    pending = None
                    for ki in range(kis):
                        if ki < nfull:
                            j, qoff, w = -1, 0, QBLK
                        else:
                            j = ki - nfull
                            qoff = j * P
                            w = QBLK - qoff
                        qg = q0 + qs + qoff
                        kg = q0 + ki * P
                        ks = slice(kg, kg + P)
                        sc = psA.tile([P, QBLK], F32, tag="psA", name="sc")
                        nc.tensor.matmul(
                            sc[:, :w], lhsT=kn_sb[h][:, ks],
                            rhs=qn_sb[h][:, qg:qg + w], start=True, stop=False,
                        )
                        nc.tensor.matmul(
                            sc[:, :w], lhsT=kr_sb[:, ks],
                            rhs=qr_sb[h][:, qg:qg + w], start=False, stop=True,
                        )
                        if ki >= nfull:
                            # additive causal mask on raw scores in PSUM; only
                            # the leading 128 cols contain the triangle, all
                            # columns beyond the diagonal block are valid
                            nc.vector.tensor_add(sc[:, 0:P], sc[:, 0:P], msk[:])
                        pr = prs.tile([P, QBLK], BF16, tag="probs", name="pr")
                        nc.scalar.activation(pr[:, :w], sc[:, :w], Exp,
                                             scale=SCALE, bias=ebias[:])
                        # software pipeline: emit PV for the PREVIOUS chunk so
                        # PE streams scores(ki) while ACT computes exp(ki-1)
                        if pending is not None:
                            emit_pv(*pending)
                        pending = (ki, j, qoff, pr)
                    emit_pv(*pending)
                    oq = osb.tile([P, QBLK], F32, tag="oq", name="oq")
                    for j4 in range(QBLK // P):
                        op = ops[j4 // 2]
                        off = (j4 % 2) * DVA
                        rec = osb.tile([P, 1], F32, tag="rec", name="rec")
                        nc.vector.reciprocal(rec[:], op[:, off + DV:off + DVA])
                        nc.vector.tensor_scalar_mul(
                            oq[:, j4 * P:(j4 + 1) * P], op[:, off:off + DV], rec[:]
                        )
                    # one 256KB DMA per q-block on the ACT HWDGE ring;
                    # oq[p, j4*128+d] <-> out[h, q0+qs+j4*128+p, d]
                    nc.scalar.dma_start(
                        out[h, q0 + qs:q0 + qs + QBLK, :]
                        .rearrange("(j p) d -> p j d", p=P),
                        oq[:].rearrange("p (j d) -> p j d", d=DV),
                    )

    nc.compile()
    return nc


def _prep_inputs(q, k, w_key, w_vo):
    bf = ml_dtypes.bfloat16
    kv_c = np.ascontiguousarray(k[:, 0, :R])          # [N, 512]
    k_rope = np.ascontiguousarray(k[:, 0, R:])        # [N, 64]
    # kvt[blk][rl, c*BCOLS+nl] = kv_c[blk*BCOLS+nl, c*128+rl]
    kvt = np.ascontiguousarray(
        kv_c.T.reshape(NRC, P, NBLK, BCOLS).transpose(2, 1, 0, 3)
        .reshape(NBLK, P, NRC * BCOLS).astype(bf))
    krt = np.ascontiguousarray(k_rope.T.astype(bf))   # [64, N]
    msk = np.where(np.triu(np.ones((P, P), bool)), 0.0, -1e5).astype(np.float32)

    in_maps = []
    for core in range(NCORES):
        hs = slice(core * HPC, (core + 1) * HPC)
        qh = q[:, hs, :]                              # [N, HPC, 192]
        qtn = np.ascontiguousarray(
            qh[:, :, :DN].transpose(1, 2, 0).astype(bf))   # [HPC, 128, N]
        qtr = np.ascontiguousarray(
            qh[:, :, DN:].transpose(1, 2, 0).astype(bf))   # [HPC, 64, N]
        # wkt[h][rl, c*DN+d] = w_key[hs][h, d, c*128+rl]
        wkt = np.ascontiguousarray(
            w_key[hs].transpose(0, 2, 1).reshape(HPC, NRC, P, DN)
            .transpose(0, 2, 1, 3).reshape(HPC, P, NRC * DN).astype(bf))
        # wvt[c][rl, h*DV + d] = w_vo[hs][h, d, c*128+rl]
        wvt = np.ascontiguousarray(
            w_vo[hs].transpose(2, 0, 1)               # [512r, HPC, 128d]
            .reshape(NRC, P, HPC * DV).astype(bf))
        in_maps.append({
            "qtn": qtn, "qtr": qtr, "kvt": kvt, "krt": krt,
            "wkt": wkt, "wvt": wvt, "mskd": msk,
        })
    return in_maps


def run(q, k, v, w_key, w_vo, trace=False, tmpdir=None):
    """Returns (output [N, H, 128] f32, BassKernelResults)."""
    if "nc" not in _CACHE:
        _CACHE["nc"] = _build()
    nc = _CACHE["nc"]
    in_maps = _prep_inputs(np.asarray(q), np.asarray(k),
                           np.asarray(w_key), np.asarray(w_vo))
    res = run_bass_kernel_spmd(
        nc, in_maps, core_ids=list(range(NCORES)), trace=trace, tmpdir=tmpdir
    )
    outs = [np.asarray(res.results[i]["out"], dtype=np.float32)
            for i in range(NCORES)]
    full = np.concatenate(outs, axis=0)               # [16, N, 128]
    return np.ascontiguousarray(full.transpose(1, 0, 2)), res


def kernel(q, k, v, w_key, w_vo):
    return run(q, k, v, w_key, w_vo)[0]
